# revision 4
# baseline (speedup 1.0000x reference)
"""Trainium2 Bass kernel for nn_DenseAttention (linear attention, no softmax).

Math (reassociated — the attention is fully linear, so the O(S^2) pre/attn
einsums collapse through a per-(b,q) Gram matrix):

    x  = hidden_states.reshape(b, t, s, h)
    G[b,q]    = x[b,:,q,:]^T @ x[b,:,q,:]                   # [h, h]
    Mf[b,a]   = sum_q qw[a,:,q,:] @ G[b,q] @ C[a, q*h:(q+1)*h, :]
    out[b,:,a*h:(a+1)*h] = x[b,:,a,:] @ Mf[b,a]

Sharding: 8 cores = (b in 0..1) x (a in 0..3). Each core streams x[b]
once for the Gram stage, computes its own Mf[b,a], and produces the
[2048, 256] output slice out[b, :, a*h:(a+1)*h]. Gather is concatenation.
"""

import os
import numpy as np

import concourse.bass as bass
import concourse.mybir as mybir
import concourse.tile as tile
from concourse import bacc
from concourse.bass_utils import run_bass_kernel_spmd

BS, S, E = 2, 2048, 1024
SQ, H = 4, 256  # sqrt_n_heads, head_size
P = 128
DT = mybir.dt.float32

_PROGRAM = None
LAST_RESULTS = None  # test harness reads exec_time_ns from here


def _build_program():
    nc = bacc.Bacc("TRN2", target_bir_lowering=False, debug=False)

    xb = nc.dram_tensor("xb", [S, E], DT, kind="ExternalInput").ap()
    xaT = nc.dram_tensor("xaT", [H, S], DT, kind="ExternalInput").ap()
    qwT = nc.dram_tensor("qwT", [SQ, H, H], DT, kind="ExternalInput").ap()
    cmb = nc.dram_tensor("cmb", [SQ * H, H], DT, kind="ExternalInput").ap()
    out = nc.dram_tensor("out", [S, H], DT, kind="ExternalOutput").ap()

    NT = S // P  # 16 row chunks of x

    with tile.TileContext(nc) as tc:
        with (
            tc.tile_pool(name="xs", bufs=4) as xs_pool,
            tc.tile_pool(name="consts", bufs=1) as const_pool,
            tc.tile_pool(name="ps", bufs=8, space="PSUM") as ps_pool,
            tc.tile_pool(name="osb", bufs=4) as out_pool,
        ):
            # Small operands (overlap with the x stream).
            xaT_sb = const_pool.tile([P, 2, S], DT, tag="xaT")
            nc.sync.dma_start(out=xaT_sb[:], in_=xaT.rearrange("(ec p) t -> p ec t", p=P))
            qwT_sb = const_pool.tile([P, SQ, 2, H], DT, tag="qwT")
            nc.sync.dma_start(out=qwT_sb[:], in_=qwT.rearrange("q (fc p) e -> p q fc e", fc=2, p=P))
            c_sb = const_pool.tile([P, SQ, 2, H], DT, tag="cmb")
            nc.sync.dma_start(out=c_sb[:], in_=cmb.rearrange("(q gc p) g -> p q gc g", q=SQ, gc=2, p=P))

            # Phase A: G[q] (q=0..3) accumulated in PSUM over the t stream.
            # g_ps[q*2+fc][p, g] accumulates G[q][fc*128+p, g].
            g_ps = [ps_pool.tile([P, H], DT, tag="ps", name=f"g_ps{i}") for i in range(8)]
            for ti in range(NT):
                xt = xs_pool.tile([P, E], DT, tag="xt")
                nc.sync.dma_start(out=xt[:], in_=xb[ti * P:(ti + 1) * P, :])
                for q in range(SQ):
                    for fc in range(2):
                        nc.tensor.matmul(
                            g_ps[q * 2 + fc][:],
                            xt[:, q * H + fc * P: q * H + fc * P + P],
                            xt[:, q * H:(q + 1) * H],
                            start=(ti == 0),
                            stop=(ti == NT - 1),
                        )
            # G is symmetric: g_sb[p, q, i, g] = G[q][i*128+p, g] can be read
            # with the partition axis as either f or g.
            g_sb = const_pool.tile([P, SQ, 2, H], DT, tag="gsb")
            for i in range(8):
                nc.vector.tensor_copy(g_sb[:, i // 2, i % 2, :], g_ps[i][:])

            # Phase B: T1[q] = G[q] @ C[a, q-rows, :]   ([h, h] each)
            t1_sb = const_pool.tile([P, SQ, 2, H], DT, tag="t1")
            for q in range(SQ):
                for fc in range(2):
                    t1_ps = ps_pool.tile([P, H], DT, tag="ps")
                    for gc in range(2):
                        nc.tensor.matmul(
                            t1_ps[:],
                            g_sb[:, q, gc, fc * P:(fc + 1) * P],
                            c_sb[:, q, gc, :],
                            start=(gc == 0),
                            stop=(gc == 1),
                        )
                    nc.vector.tensor_copy(t1_sb[:, q, fc, :], t1_ps[:])

            # Phase C: Mf = sum_q qw_aq @ T1[q]  ([h(e), h(g2)], e-partitioned)
            mf_sb = const_pool.tile([P, 2, H], DT, tag="mf")
            for ec in range(2):
                mf_ps = ps_pool.tile([P, H], DT, tag="ps")
                k = 0
                for q in range(SQ):
                    for fc in range(2):
                        nc.tensor.matmul(
                            mf_ps[:],
                            qwT_sb[:, q, fc, ec * P:(ec + 1) * P],
                            t1_sb[:, q, fc, :],
                            start=(k == 0),
                            stop=(k == 7),
                        )
                        k += 1
                nc.vector.tensor_copy(mf_sb[:, ec, :], mf_ps[:])

            # Phase D: out rows = x[b,:,a,:] @ Mf
            for ti in range(NT):
                o_ps = ps_pool.tile([P, H], DT, tag="ps")
                for ec in range(2):
                    nc.tensor.matmul(
                        o_ps[:],
                        xaT_sb[:, ec, ti * P:(ti + 1) * P],
                        mf_sb[:, ec, :],
                        start=(ec == 0),
                        stop=(ec == 1),
                    )
                o_sb = out_pool.tile([P, H], DT, tag="osb")
                nc.vector.tensor_copy(o_sb[:], o_ps[:])
                nc.sync.dma_start(out=out[ti * P:(ti + 1) * P, :], in_=o_sb[:])

    nc.compile()
    return nc


def _get_program():
    global _PROGRAM
    if _PROGRAM is None:
        _PROGRAM = _build_program()
    return _PROGRAM


def _make_in_maps(hidden_states, queries, combiners):
    x = np.ascontiguousarray(np.asarray(hidden_states, dtype=np.float32))
    qs = np.asarray(queries, dtype=np.float32)
    cb = np.asarray(combiners, dtype=np.float32)
    in_maps = []
    for c in range(8):
        b, a = divmod(c, 4)
        in_maps.append({
            "xb": x[b],
            "xaT": np.ascontiguousarray(x[b][:, a * H:(a + 1) * H].T),
            # qwT[q][f, e] = qw[a, e, q, f]
            "qwT": np.ascontiguousarray(qs[a].reshape(H, SQ, H).transpose(1, 2, 0)),
            "cmb": np.ascontiguousarray(cb[a]),
        })
    return in_maps


def kernel(hidden_states, queries, combiners):
    global LAST_RESULTS
    nc = _get_program()
    in_maps = _make_in_maps(hidden_states, queries, combiners)
    res = run_bass_kernel_spmd(
        nc, in_maps, core_ids=list(range(8)),
        trace=bool(os.environ.get("BASS_TRACE")),
    )
    LAST_RESULTS = res
    out = np.empty((BS, S, E), dtype=np.float32)
    for c in range(8):
        b, a = divmod(c, 4)
        out[b, :, a * H:(a + 1) * H] = res.results[c]["out"]
    return out


# revision 11
# speedup vs baseline: 1.6095x; 1.6095x over previous
"""Trainium2 Bass kernel for nn_DenseAttention (linear attention, no softmax).

Math (reassociated — the attention is fully linear, so the O(S^2) pre/attn
einsums collapse through a per-(b,q) Gram matrix):

    x  = hidden_states.reshape(b, t, s, h)
    G[b,q]    = x[b,:,q,:]^T @ x[b,:,q,:]                   # [h, h]
    Mf[b,a]   = sum_q qw[a,:,q,:] @ G[b,q] @ C[a, q*h:(q+1)*h, :]
    out[b,:,a*h:(a+1)*h] = x[b,:,a,:] @ Mf[b,a]

Sharding: 8 cores = (b in 0..1) x (a in 0..3). Each core streams x[b]
once for the Gram stage, computes its own Mf[b,a], and produces the
[2048, 256] output slice out[b, :, a*h:(a+1)*h]. Gather is concatenation.
"""

import os
import numpy as np

import concourse.bass as bass
import concourse.mybir as mybir
import concourse.tile as tile
from concourse import bacc
from concourse.bass_utils import run_bass_kernel_spmd

BS, S, E = 2, 2048, 1024
SQ, H = 4, 256  # sqrt_n_heads, head_size
P = 128
DT = mybir.dt.float32
# float32r: single-pass PE fp32 mode (1 cycle/row for N>=256 vs 4 for the
# two-pass LOW_HIGH fp32). Same 4-byte layout; matmul operands carry this
# dtype end-to-end so the BIR verifier sees every producer as f32r.
DTR = mybir.dt.float32r


def _mm(nc, out, lhsT, rhs, start, stop):
    nc.tensor.matmul(out, lhsT, rhs, start=start, stop=stop)

_PROGRAM = None
LAST_RESULTS = None  # test harness reads exec_time_ns from here


def _build_program():
    nc = bacc.Bacc("TRN2", target_bir_lowering=False, debug=False)

    xb = nc.dram_tensor("xb", [S, E], DTR, kind="ExternalInput").ap()
    xaT = nc.dram_tensor("xaT", [H, S], DTR, kind="ExternalInput").ap()
    qwT = nc.dram_tensor("qwT", [SQ, H, H], DTR, kind="ExternalInput").ap()
    cmb = nc.dram_tensor("cmb", [SQ * H, H], DTR, kind="ExternalInput").ap()
    out = nc.dram_tensor("out", [S, H], DT, kind="ExternalOutput").ap()

    NT = S // P  # 16 row chunks of x

    with tile.TileContext(nc) as tc:
        with (
            tc.tile_pool(name="xs", bufs=4) as xs_pool,
            tc.tile_pool(name="consts", bufs=1) as const_pool,
            tc.tile_pool(name="ps", bufs=8, space="PSUM") as ps_pool,
            tc.tile_pool(name="osb", bufs=4) as out_pool,
        ):
            # Small operands (overlap with the x stream).
            xaT_sb = const_pool.tile([P, 2, S], DTR, tag="xaT")
            nc.sync.dma_start(out=xaT_sb[:], in_=xaT.rearrange("(ec p) t -> p ec t", p=P))
            qwT_sb = const_pool.tile([P, SQ, 2, H], DTR, tag="qwT")
            nc.sync.dma_start(out=qwT_sb[:], in_=qwT.rearrange("q (fc p) e -> p q fc e", fc=2, p=P))
            c_sb = const_pool.tile([P, SQ, 2, H], DTR, tag="cmb")
            nc.sync.dma_start(out=c_sb[:], in_=cmb.rearrange("(q gc p) g -> p q gc g", q=SQ, gc=2, p=P))

            # Phase A: G[q] (q=0..3) accumulated in PSUM over the t stream.
            # g_ps[q*2+fc][p, g] accumulates G[q][fc*128+p, g].
            g_ps = [ps_pool.tile([P, H], DT, tag="ps", name=f"g_ps{i}") for i in range(8)]
            for ti in range(NT):
                xt = xs_pool.tile([P, E], DTR, tag="xt")
                nc.sync.dma_start(out=xt[:], in_=xb[ti * P:(ti + 1) * P, :])
                for q in range(SQ):
                    for fc in range(2):
                        _mm(
                            nc,
                            g_ps[q * 2 + fc][:],
                            xt[:, q * H + fc * P: q * H + fc * P + P],
                            xt[:, q * H:(q + 1) * H],
                            start=(ti == 0),
                            stop=(ti == NT - 1),
                        )
            # G is symmetric: g_sb[p, q, i, g] = G[q][i*128+p, g] can be read
            # with the partition axis as either f or g.
            g_sb = const_pool.tile([P, SQ, 2, H], DTR, tag="gsb")
            for i in range(8):
                nc.vector.tensor_copy(g_sb[:, i // 2, i % 2, :], g_ps[i][:])

            # Phase B: T1[q] = G[q] @ C[a, q-rows, :]   ([h, h] each)
            t1_sb = const_pool.tile([P, SQ, 2, H], DTR, tag="t1")
            for q in range(SQ):
                for fc in range(2):
                    t1_ps = ps_pool.tile([P, H], DT, tag="ps")
                    for gc in range(2):
                        _mm(
                            nc,
                            t1_ps[:],
                            g_sb[:, q, gc, fc * P:(fc + 1) * P],
                            c_sb[:, q, gc, :],
                            start=(gc == 0),
                            stop=(gc == 1),
                        )
                    nc.vector.tensor_copy(t1_sb[:, q, fc, :], t1_ps[:])

            # Phase C: Mf = sum_q qw_aq @ T1[q]  ([h(e), h(g2)], e-partitioned)
            mf_sb = const_pool.tile([P, 2, H], DTR, tag="mf")
            for ec in range(2):
                mf_ps = ps_pool.tile([P, H], DT, tag="ps")
                k = 0
                for q in range(SQ):
                    for fc in range(2):
                        _mm(
                            nc,
                            mf_ps[:],
                            qwT_sb[:, q, fc, ec * P:(ec + 1) * P],
                            t1_sb[:, q, fc, :],
                            start=(k == 0),
                            stop=(k == 7),
                        )
                        k += 1
                nc.vector.tensor_copy(mf_sb[:, ec, :], mf_ps[:])

            # Phase D: out rows = x[b,:,a,:] @ Mf
            for ti in range(NT):
                o_ps = ps_pool.tile([P, H], DT, tag="ps")
                for ec in range(2):
                    _mm(
                        nc,
                        o_ps[:],
                        xaT_sb[:, ec, ti * P:(ti + 1) * P],
                        mf_sb[:, ec, :],
                        start=(ec == 0),
                        stop=(ec == 1),
                    )
                o_sb = out_pool.tile([P, H], DT, tag="osb")
                nc.vector.tensor_copy(o_sb[:], o_ps[:])
                nc.sync.dma_start(out=out[ti * P:(ti + 1) * P, :], in_=o_sb[:])

    nc.compile()
    return nc


def _get_program():
    global _PROGRAM
    if _PROGRAM is None:
        _PROGRAM = _build_program()
    return _PROGRAM


def _make_in_maps(hidden_states, queries, combiners):
    x = np.ascontiguousarray(np.asarray(hidden_states, dtype=np.float32))
    qs = np.asarray(queries, dtype=np.float32)
    cb = np.asarray(combiners, dtype=np.float32)
    in_maps = []
    for c in range(8):
        b, a = divmod(c, 4)
        in_maps.append({
            "xb": x[b],
            "xaT": np.ascontiguousarray(x[b][:, a * H:(a + 1) * H].T),
            # qwT[q][f, e] = qw[a, e, q, f]
            "qwT": np.ascontiguousarray(qs[a].reshape(H, SQ, H).transpose(1, 2, 0)),
            "cmb": np.ascontiguousarray(cb[a]),
        })
    return in_maps


def kernel(hidden_states, queries, combiners):
    global LAST_RESULTS
    nc = _get_program()
    in_maps = _make_in_maps(hidden_states, queries, combiners)
    res = run_bass_kernel_spmd(
        nc, in_maps, core_ids=list(range(8)),
        trace=bool(os.environ.get("BASS_TRACE")),
    )
    LAST_RESULTS = res
    out = np.empty((BS, S, E), dtype=np.float32)
    for c in range(8):
        b, a = divmod(c, 4)
        out[b, :, a * H:(a + 1) * H] = res.results[c]["out"]
    return out


# revision 12
# speedup vs baseline: 1.6437x; 1.0213x over previous
"""Trainium2 Bass kernel for nn_DenseAttention (linear attention, no softmax).

Math (reassociated — the attention is fully linear, so the O(S^2) pre/attn
einsums collapse through a per-(b,q) Gram matrix):

    x  = hidden_states.reshape(b, t, s, h)
    G[b,q]    = x[b,:,q,:]^T @ x[b,:,q,:]                   # [h, h]
    Mf[b,a]   = sum_q qw[a,:,q,:] @ G[b,q] @ C[a, q*h:(q+1)*h, :]
    out[b,:,a*h:(a+1)*h] = x[b,:,a,:] @ Mf[b,a]

Sharding: 8 cores = (b in 0..1) x (a in 0..3). Each core streams x[b]
once for the Gram stage, computes its own Mf[b,a], and produces the
[2048, 256] output slice out[b, :, a*h:(a+1)*h]. Gather is concatenation.
"""

import os
import numpy as np

import concourse.bass as bass
import concourse.mybir as mybir
import concourse.tile as tile
from concourse import bacc
from concourse.bass_utils import run_bass_kernel_spmd

BS, S, E = 2, 2048, 1024
SQ, H = 4, 256  # sqrt_n_heads, head_size
P = 128
DT = mybir.dt.float32
# float32r: single-pass PE fp32 mode (1 cycle/row for N>=256 vs 4 for the
# two-pass LOW_HIGH fp32). Same 4-byte layout; matmul operands carry this
# dtype end-to-end so the BIR verifier sees every producer as f32r.
DTR = mybir.dt.float32r


def _mm(nc, out, lhsT, rhs, start, stop):
    nc.tensor.matmul(out, lhsT, rhs, start=start, stop=stop)

_PROGRAM = None
LAST_RESULTS = None  # test harness reads exec_time_ns from here


def _build_program():
    nc = bacc.Bacc("TRN2", target_bir_lowering=False, debug=False)

    xb = nc.dram_tensor("xb", [S, E], DTR, kind="ExternalInput").ap()
    xaT = nc.dram_tensor("xaT", [H, S], DTR, kind="ExternalInput").ap()
    qwT = nc.dram_tensor("qwT", [SQ, H, H], DTR, kind="ExternalInput").ap()
    cmb = nc.dram_tensor("cmb", [SQ * H, H], DTR, kind="ExternalInput").ap()
    out = nc.dram_tensor("out", [S, H], DT, kind="ExternalOutput").ap()

    NT = S // P  # 16 row chunks of x

    with tile.TileContext(nc) as tc:
        with (
            tc.tile_pool(name="xs", bufs=4) as xs_pool,
            tc.tile_pool(name="consts", bufs=1) as const_pool,
            tc.tile_pool(name="ps", bufs=8, space="PSUM") as ps_pool,
            tc.tile_pool(name="osb", bufs=4) as out_pool,
        ):
            # Allocate small-operand tiles; their DMAs are issued mid x-stream
            # so they don't delay phase A's first chunks in the DMA queues.
            xaT_sb = const_pool.tile([P, 2, S], DTR, tag="xaT")
            qwT_sb = const_pool.tile([P, SQ, 2, H], DTR, tag="qwT")
            c_sb = const_pool.tile([P, SQ, 2, H], DTR, tag="cmb")

            # Phase A: G[q] (q=0..3) accumulated in PSUM over the t stream.
            # g_ps[q*2+fc][p, g] accumulates G[q][fc*128+p, g].
            g_ps = [ps_pool.tile([P, H], DT, tag="ps", name=f"g_ps{i}") for i in range(8)]
            for ti in range(NT):
                xt = xs_pool.tile([P, E], DTR, tag="xt")
                nc.sync.dma_start(out=xt[:], in_=xb[ti * P:(ti + 1) * P, :])
                if ti == 2:
                    nc.sync.dma_start(
                        out=qwT_sb[:],
                        in_=qwT.rearrange("q (fc p) e -> p q fc e", fc=2, p=P),
                    )
                    nc.sync.dma_start(
                        out=c_sb[:],
                        in_=cmb.rearrange("(q gc p) g -> p q gc g", q=SQ, gc=2, p=P),
                    )
                elif ti == 4:
                    nc.sync.dma_start(
                        out=xaT_sb[:], in_=xaT.rearrange("(ec p) t -> p ec t", p=P)
                    )
                for q in range(SQ):
                    for fc in range(2):
                        _mm(
                            nc,
                            g_ps[q * 2 + fc][:],
                            xt[:, q * H + fc * P: q * H + fc * P + P],
                            xt[:, q * H:(q + 1) * H],
                            start=(ti == 0),
                            stop=(ti == NT - 1),
                        )
            # G is symmetric: g_sb[p, q, i, g] = G[q][i*128+p, g] can be read
            # with the partition axis as either f or g.
            g_sb = const_pool.tile([P, SQ, 2, H], DTR, tag="gsb")
            for i in range(8):
                nc.vector.tensor_copy(g_sb[:, i // 2, i % 2, :], g_ps[i][:])

            # Phase B: T1[q] = G[q] @ C[a, q-rows, :]   ([h, h] each)
            t1_sb = const_pool.tile([P, SQ, 2, H], DTR, tag="t1")
            for q in range(SQ):
                for fc in range(2):
                    t1_ps = ps_pool.tile([P, H], DT, tag="ps")
                    for gc in range(2):
                        _mm(
                            nc,
                            t1_ps[:],
                            g_sb[:, q, gc, fc * P:(fc + 1) * P],
                            c_sb[:, q, gc, :],
                            start=(gc == 0),
                            stop=(gc == 1),
                        )
                    nc.vector.tensor_copy(t1_sb[:, q, fc, :], t1_ps[:])

            # Phase C: Mf = sum_q qw_aq @ T1[q]  ([h(e), h(g2)], e-partitioned)
            mf_sb = const_pool.tile([P, 2, H], DTR, tag="mf")
            for ec in range(2):
                mf_ps = ps_pool.tile([P, H], DT, tag="ps")
                k = 0
                for q in range(SQ):
                    for fc in range(2):
                        _mm(
                            nc,
                            mf_ps[:],
                            qwT_sb[:, q, fc, ec * P:(ec + 1) * P],
                            t1_sb[:, q, fc, :],
                            start=(k == 0),
                            stop=(k == 7),
                        )
                        k += 1
                nc.vector.tensor_copy(mf_sb[:, ec, :], mf_ps[:])

            # Phase D: out rows = x[b,:,a,:] @ Mf
            for ti in range(NT):
                o_ps = ps_pool.tile([P, H], DT, tag="ps")
                for ec in range(2):
                    _mm(
                        nc,
                        o_ps[:],
                        xaT_sb[:, ec, ti * P:(ti + 1) * P],
                        mf_sb[:, ec, :],
                        start=(ec == 0),
                        stop=(ec == 1),
                    )
                o_sb = out_pool.tile([P, H], DT, tag="osb")
                nc.vector.tensor_copy(o_sb[:], o_ps[:])
                nc.sync.dma_start(out=out[ti * P:(ti + 1) * P, :], in_=o_sb[:])

    nc.compile()
    return nc


def _get_program():
    global _PROGRAM
    if _PROGRAM is None:
        _PROGRAM = _build_program()
    return _PROGRAM


def _make_in_maps(hidden_states, queries, combiners):
    x = np.ascontiguousarray(np.asarray(hidden_states, dtype=np.float32))
    qs = np.asarray(queries, dtype=np.float32)
    cb = np.asarray(combiners, dtype=np.float32)
    in_maps = []
    for c in range(8):
        b, a = divmod(c, 4)
        in_maps.append({
            "xb": x[b],
            "xaT": np.ascontiguousarray(x[b][:, a * H:(a + 1) * H].T),
            # qwT[q][f, e] = qw[a, e, q, f]
            "qwT": np.ascontiguousarray(qs[a].reshape(H, SQ, H).transpose(1, 2, 0)),
            "cmb": np.ascontiguousarray(cb[a]),
        })
    return in_maps


def kernel(hidden_states, queries, combiners):
    global LAST_RESULTS
    nc = _get_program()
    in_maps = _make_in_maps(hidden_states, queries, combiners)
    res = run_bass_kernel_spmd(
        nc, in_maps, core_ids=list(range(8)),
        trace=bool(os.environ.get("BASS_TRACE")),
    )
    LAST_RESULTS = res
    out = np.empty((BS, S, E), dtype=np.float32)
    for c in range(8):
        b, a = divmod(c, 4)
        out[b, :, a * H:(a + 1) * H] = res.results[c]["out"]
    return out
